# revision 43
# baseline (speedup 1.0000x reference)
"""Trainium2 Bass kernel for a small dense transformer block.

Model (per reference):
  x   : [B, T, D]  B=16, T=2048, D=40, H=4 heads, hs=10
  ln1 -> per-head q/k/v -> scores = k @ q^T (softmax over q index) -> out @ Wp
  residual (on ln1(x)) -> ln2 -> FFN(relu) -> residual (on ln2 output)

Sharding: data-parallel over batch, 2 batches per core across 8 cores.
Layout strategy: feature-major ("transposed", [D, T]) activations so every
activation/weight matmul has its contraction dim on partitions; per-token
scalars (softmax 1/Z, LN mean/rstd) are broadcast across partitions with
tiny PE matmuls. Heads packed at partition offsets 32h to use tile_position
row/col packing (head_size=10 << 128).
"""

import sys
from contextlib import ExitStack

for _p in ("/opt/trn_rl_repo",):
    if _p not in sys.path:
        sys.path.insert(0, _p)

import numpy as np

import concourse.bass as bass
import concourse.tile as tile
from concourse import mybir
from concourse.masks import make_identity

B_FULL = 16
N_CORES = 8
B_LOC = B_FULL // N_CORES
T = 2048
D = 40
H = 4
HS = 10
LN_EPS = 1e-5

F32 = mybir.dt.float32
F32R = mybir.dt.float32r
BF16 = mybir.dt.bfloat16
AF = mybir.ActivationFunctionType
OP = mybir.AluOpType
PIPELINE_PV = True


def build_kernel(b_loc=B_LOC, t_len=T, split_waits=True):
    nc = bass.Bass("TRN2", target_bir_lowering=False)

    x_d = nc.dram_tensor("x", [b_loc, t_len, D], F32, kind="ExternalInput")
    wq_d = nc.dram_tensor("Wq", [H, D, HS], F32, kind="ExternalInput")
    wk_d = nc.dram_tensor("Wk", [H, D, HS], F32, kind="ExternalInput")
    wv_d = nc.dram_tensor("Wv", [H, D, HS], F32, kind="ExternalInput")
    wp_d = nc.dram_tensor("Wp", [D, D], F32, kind="ExternalInput")
    bp_d = nc.dram_tensor("bp", [D], F32, kind="ExternalInput")
    w1_d = nc.dram_tensor("W1", [D, D], F32, kind="ExternalInput")
    b1_d = nc.dram_tensor("b1", [D], F32, kind="ExternalInput")
    w2_d = nc.dram_tensor("W2", [D, D], F32, kind="ExternalInput")
    b2_d = nc.dram_tensor("b2", [D], F32, kind="ExternalInput")
    g1_d = nc.dram_tensor("g1", [D], F32, kind="ExternalInput")
    be1_d = nc.dram_tensor("be1", [D], F32, kind="ExternalInput")
    g2_d = nc.dram_tensor("g2", [D], F32, kind="ExternalInput")
    be2_d = nc.dram_tensor("be2", [D], F32, kind="ExternalInput")
    out_d = nc.dram_tensor("out", [b_loc, t_len, D], F32, kind="ExternalOutput")

    n_tt = t_len // 128                     # token tiles
    IC = 512 if t_len % 512 == 0 else t_len     # attention i-chunk width
    n_ic = t_len // IC
    SC = min(512, IC)                           # post-attention subchunk width
    n_sc = IC // SC
    NMA = min(512, t_len)                   # q/k projection chunk

    with tile.TileContext(nc) as tc, ExitStack() as ctx:
        consts = ctx.enter_context(tc.tile_pool(name="consts", bufs=1))

        iden = consts.tile([128, 128], F32)
        make_identity(nc, iden)

        eps128 = consts.tile([128, 1], F32)
        nc.vector.memset(eps128, LN_EPS)

        # [40, 128] q/k weights, head h at columns 32h..32h+9
        def load_wqk(w_dram, name):
            t_ = consts.tile([D, 128], F32, tag=name)
            nc.vector.memset(t_, 0.0)
            dst = t_[:].rearrange("p (h w) -> p h w", w=32)[:, :, 0:HS]
            nc.sync.dma_start(out=dst, in_=w_dram[:].transpose([1, 0, 2]))
            return t_

        wqT = load_wqk(wq_d, "wqT")
        wkT = load_wqk(wk_d, "wkT")
        # v weights [40, 128]: head h at cols 32h..32h+9; col 32h+10 stays 0
        # (later memset to carry the ones column for the softmax denominator)
        wv128 = load_wqk(wv_d, "wv128")

        # Wp packed [128, 40]: row 32h+e = Wp[10h+e, :]; other rows zero
        wpp = consts.tile([128, D], F32)
        nc.vector.memset(wpp, 0.0)
        for h in range(H):
            nc.sync.dma_start(
                out=wpp[32 * h : 32 * h + HS, :],
                in_=wp_d[HS * h : HS * h + HS, :],
            )

        w1s = consts.tile([D, D], F32)
        nc.sync.dma_start(out=w1s, in_=w1_d[:])
        w2s = consts.tile([D, D], F32)
        nc.sync.dma_start(out=w2s, in_=w2_d[:])

        def load_col(v_dram, name):
            t_ = consts.tile([D, 1], F32, tag=name)
            nc.sync.dma_start(out=t_, in_=v_dram[:].unsqueeze(1))
            return t_

        bpc = load_col(bp_d, "bpc")
        b1c = load_col(b1_d, "b1c")
        b2c = load_col(b2_d, "b2c")
        g2c = load_col(g2_d, "g2c")
        be2c = load_col(be2_d, "be2c")

        # g1/be1 broadcast across 128 partitions (token-major LN1)
        def load_bc(v_dram, name):
            t_ = consts.tile([128, D], F32, tag=name)
            v_ap = v_dram[:]
            src = bass.AP(tensor=v_ap.tensor, offset=v_ap.offset,
                          ap=[[0, 128], [1, D]])
            nc.sync.dma_start(out=t_, in_=src)
            return t_

        g1bc = load_bc(g1_d, "g1bc")
        be1bc = load_bc(be1_d, "be1bc")

        # sel [128,128]: row 32h+10 has ones in cols 32h..32h+31 -> Z broadcast
        # built via affine predicate p - 32*g - 10 == 0 on the [128,4,32] view
        sel = consts.tile([128, 128], F32)
        nc.gpsimd.memset(sel, 0.0)
        nc.gpsimd.affine_select(
            out=sel[:].rearrange("p (g w) -> p g w", w=32),
            in_=sel[:].rearrange("p (g w) -> p g w", w=32),
            compare_op=OP.not_equal,
            fill=1.0,
            base=-HS,
            pattern=[[-32, 4], [0, 32]],
            channel_multiplier=1,
        )

        ones128 = consts.tile([128, 1], F32)
        nc.vector.memset(ones128, 1.0)

        # ones/D column [40,1] for LN2 mean matmuls
        onesD = consts.tile([D, 1], F32)
        nc.vector.memset(onesD, 1.0 / D)
        # ones row [1,40] for LN2 stat broadcast
        ones1 = consts.tile([1, D], F32)
        nc.vector.memset(ones1, 1.0)

        # ---------------- per-batch persistent SBUF ----------------
        persist = ctx.enter_context(tc.tile_pool(name="persist", bufs=1))
        xnT = [persist.tile([D, t_len], F32, tag=f"xnT{b}", name=f"xnT{b}")
               for b in range(b_loc)]
        qT = [persist.tile([128, t_len], BF16, tag=f"qT{b}", name=f"qT{b}")
              for b in range(b_loc)]
        kT = [persist.tile([128, t_len], BF16, tag=f"kT{b}", name=f"kT{b}")
              for b in range(b_loc)]
        vA = [persist.tile([128, n_tt, 128], BF16, tag=f"vA{b}", name=f"vA{b}")
              for b in range(b_loc)]

        # ================= stage A: LN1 + transpose + QKV =================
        sbA = ctx.enter_context(tc.tile_pool(name="sbA", bufs=4))
        xtp = ctx.enter_context(tc.tile_pool(name="xtp", bufs=n_tt + 1))

        # ================= stage B+C: attention + tail =================
        with (
            tc.tile_pool(name="spool", bufs=2, space="PSUM") as sp,
            tc.tile_pool(name="pvpool", bufs=1, space="PSUM") as pvp,
            tc.tile_pool(name="psC", bufs=2, space="PSUM") as pC,
            tc.tile_pool(name="psA", bufs=1, space="PSUM") as pA,
            tc.tile_pool(name="epool", bufs=6) as ep,
            tc.tile_pool(name="sbC", bufs=2) as sC,
            tc.tile_pool(name="outp", bufs=4) as op_,
        ):
            sA, xP = sbA, xtp

            def emit_stage_a_slices(b):
                """LN1 + transpose + QKV for batch b as a list of small
                closures so they can be dripped into the attention stream."""
                mv = persist.tile([128, n_tt, 2], F32, tag=f"mv{b}", name=f"mv{b}")
                rstd = persist.tile([128, n_tt], F32, tag=f"rstd{b}",
                                    name=f"rstd{b}")
                GRP = min(2, n_tt)
                slices = []

                def ln_group(g0, b=b, mv=mv, rstd=rstd):
                    xts = {}
                    for t_i in range(g0, g0 + GRP):
                        xt = xP.tile([128, D], F32, tag="xt", name="xt")
                        nc.sync.dma_start(
                            out=xt, in_=x_d[b, t_i * 128 : (t_i + 1) * 128, :])
                        st6 = sA.tile([128, 6], F32, tag="st6", name="st6")
                        nc.vector.bn_stats(out=st6, in_=xt)
                        nc.vector.bn_aggr(out=mv[:, t_i, :], in_=st6)
                        xts[t_i] = xt
                    lnv = sA.tile([128, GRP], F32, tag="lnv", name="lnv")
                    nc.scalar.activation(out=lnv, in_=mv[:, g0 : g0 + GRP, 1],
                                         func=AF.Ln, bias=eps128, scale=1.0)
                    nc.scalar.activation(out=rstd[:, g0 : g0 + GRP], in_=lnv,
                                         func=AF.Exp, bias=0.0, scale=-0.5)
                    for t_i in range(g0, g0 + GRP):
                        xn = sA.tile([128, D], F32, tag="xn", name="xn")
                        nc.vector.tensor_scalar(
                            out=xn, in0=xts[t_i],
                            scalar1=mv[:, t_i, 0:1],
                            scalar2=rstd[:, t_i : t_i + 1],
                            op0=OP.subtract, op1=OP.mult)
                        nc.vector.tensor_mul(out=xn, in0=xn, in1=g1bc)
                        nc.vector.tensor_add(out=xn, in0=xn, in1=be1bc)
                        tp = pA.tile([D, 128], F32, tag="a", name="tp")
                        nc.tensor.transpose(tp, xn, iden)
                        nc.vector.tensor_copy(
                            out=xnT[b][:, t_i * 128 : (t_i + 1) * 128], in_=tp)

                for g0 in range(0, n_tt, GRP):
                    slices.append(lambda g0=g0: ln_group(g0))

                def qk_chunk(c, w, dst, b=b):
                    sl = slice(c * NMA, (c + 1) * NMA)
                    qp = pA.tile([128, NMA], F32, tag="a", name="qp")
                    nc.tensor.matmul(qp, lhsT=w, rhs=xnT[b][:, sl],
                                     start=True, stop=True)
                    nc.vector.tensor_copy(out=dst[b][:, sl], in_=qp)

                for c in range(t_len // NMA):
                    slices.append(lambda c=c: qk_chunk(c, wqT, qT))
                    slices.append(lambda c=c: qk_chunk(c, wkT, kT))

                def v_group(g0, b=b):
                    for t_i in range(g0, min(g0 + 2, n_tt)):
                        vp = pA.tile([128, 128], F32, tag="a", name="vp")
                        nc.tensor.matmul(
                            vp, lhsT=xnT[b][:, t_i * 128 : (t_i + 1) * 128],
                            rhs=wv128, start=True, stop=True)
                        nc.vector.tensor_copy(out=vA[b][:, t_i, :], in_=vp)
                        ones_ap = vA[b][:, t_i, :].rearrange(
                            "p (h w) -> p h w", w=32)[:, :, HS : HS + 1]
                        o_src = ones128[:]
                        ones_bc = bass.AP(tensor=o_src.tensor, offset=o_src.offset,
                                          ap=[o_src.ap[0], [0, H], [0, 1]])
                        nc.vector.tensor_copy(out=ones_ap, in_=ones_bc)

                for g0 in range(0, n_tt, 2):
                    slices.append(lambda g0=g0: v_group(g0))
                return slices

            for f in emit_stage_a_slices(0):
                f()
            a_queue = []
            for b2 in range(1, b_loc):
                a_queue.extend(emit_stage_a_slices(b2))
            def _make_stage_c(b, i0, pv):
                def run():
                    pv_sb = sC.tile([128, IC], F32, tag="pvsb", name="pv_sb")
                    nc.vector.tensor_copy(out=pv_sb, in_=pv)
                    on = sC.tile([128, IC], F32, tag="onorm", name="on")
                    for sc_i in range(n_sc):
                        ssl = slice(sc_i * SC, (sc_i + 1) * SC)
                        gsl = slice(i0 + sc_i * SC, i0 + (sc_i + 1) * SC)
                        zbc = pC.tile([128, SC], F32, tag="c", name="zbc")
                        nc.tensor.matmul(zbc, lhsT=sel, rhs=pv_sb[:, ssl],
                                         start=True, stop=True)
                        rbc = sC.tile([128, SC], F32, tag="rbc", name="rbc")
                        nc.vector.reciprocal(out=rbc, in_=zbc)
                        nc.vector.tensor_mul(out=on[:, ssl], in0=pv_sb[:, ssl],
                                             in1=rbc)
                        # attention projection + residual -> x1
                        yp = pC.tile([D, SC], F32, tag="c", name="yp")
                        nc.tensor.matmul(yp, lhsT=wpp, rhs=on[:, ssl],
                                         start=True, stop=True)
                        x1 = sC.tile([D, SC], F32, tag="x1", name="x1")
                        nc.vector.scalar_tensor_tensor(
                            out=x1, in0=yp, scalar=bpc, in1=xnT[b][:, gsl],
                            op0=OP.add, op1=OP.add)
                        # LN2 stats
                        sq = sC.tile([D, SC], F32, tag="sq", name="sq")
                        nc.vector.tensor_mul(out=sq, in0=x1, in1=x1)
                        mup = pC.tile([1, SC], F32, tag="c", name="mup")
                        nc.tensor.matmul(mup, lhsT=onesD, rhs=x1,
                                         start=True, stop=True)
                        m2p = pC.tile([1, SC], F32, tag="c", name="m2p")
                        nc.tensor.matmul(m2p, lhsT=onesD, rhs=sq,
                                         start=True, stop=True)
                        mus = sC.tile([1, SC], F32, tag="mus", name="mus")
                        nc.vector.tensor_copy(out=mus, in_=mup)
                        msq = sC.tile([1, SC], F32, tag="msq", name="msq")
                        nc.vector.tensor_mul(out=msq, in0=mus, in1=mus)
                        var = sC.tile([1, SC], F32, tag="var", name="var")
                        nc.vector.tensor_sub(out=var, in0=m2p, in1=msq)
                        lnv2 = sC.tile([1, SC], F32, tag="lnv2", name="lnv2")
                        nc.scalar.activation(out=lnv2, in_=var, func=AF.Ln,
                                             bias=eps128[0:1, :], scale=1.0)
                        rsd = sC.tile([1, SC], F32, tag="rsd", name="rsd")
                        nc.scalar.activation(out=rsd, in_=lnv2, func=AF.Exp,
                                             bias=0.0, scale=-0.5)
                        mubc = pC.tile([D, SC], F32, tag="c", name="mubc")
                        nc.tensor.matmul(mubc, lhsT=ones1, rhs=mus,
                                         start=True, stop=True)
                        rsbc = pC.tile([D, SC], F32, tag="c", name="rsbc")
                        nc.tensor.matmul(rsbc, lhsT=ones1, rhs=rsd,
                                         start=True, stop=True)
                        t1 = sC.tile([D, SC], F32, tag="t1", name="t1")
                        nc.vector.tensor_sub(out=t1, in0=x1, in1=mubc)
                        t2 = sC.tile([D, SC], F32, tag="t2", name="t2")
                        nc.vector.tensor_mul(out=t2, in0=t1, in1=rsbc)
                        x2 = sC.tile([D, SC], F32, tag="x2", name="x2")
                        nc.vector.tensor_scalar(
                            out=x2, in0=t2, scalar1=g2c, scalar2=be2c,
                            op0=OP.mult, op1=OP.add)
                        # FFN
                        hp_ = pC.tile([D, SC], F32, tag="c", name="hp_")
                        nc.tensor.matmul(hp_, lhsT=w1s, rhs=x2,
                                         start=True, stop=True)
                        hs_ = sC.tile([D, SC], F32, tag="hs", name="hs_")
                        nc.vector.tensor_scalar(
                            out=hs_, in0=hp_, scalar1=b1c, scalar2=0.0,
                            op0=OP.add, op1=OP.max)
                        y2p = pC.tile([D, SC], F32, tag="c", name="y2p")
                        nc.tensor.matmul(y2p, lhsT=w2s, rhs=hs_,
                                         start=True, stop=True)
                        ob = sC.tile([D, SC], F32, tag="ob", name="ob")
                        nc.vector.scalar_tensor_tensor(
                            out=ob, in0=y2p, scalar=b2c, in1=x2,
                            op0=OP.add, op1=OP.add)
                        # transpose back to token-major and store
                        for tt_i in range(SC // 128):
                            otp = pC.tile([128, D], F32, tag="c", name="otp")
                            nc.tensor.transpose(
                                otp, ob[:, tt_i * 128 : (tt_i + 1) * 128],
                                iden[0:D, 0:D])
                            osb = op_.tile([128, D], F32, tag="osb", name="osb")
                            nc.vector.tensor_copy(out=osb, in_=otp)
                            t_glob = i0 + sc_i * SC + tt_i * 128
                            nc.sync.dma_start(
                                out=out_d[b, t_glob : t_glob + 128, :], in_=osb)
                return run

            pending_c = [None]
            gstep = [0]
            last_pv = [None]     # PV emission lags S/exp by one j globally
            for b in range(b_loc):
                if b > 0:
                    while a_queue:
                        a_queue.pop(0)()
                for ic in range(n_ic):
                    i0 = ic * IC
                    isl = slice(i0, i0 + IC)
                    pv = pvp.tile([128, IC], F32, tag="pv")

                    def emit_pv(j, es, b=b, pv=pv):
                        for h in range(H):
                            e = es[h // 2]
                            nc.tensor.matmul(
                                pv[32 * h : 32 * h + 32, :],
                                lhsT=vA[b][:, j, 32 * h : 32 * h + 32],
                                rhs=e[:, h % 2, 0:IC],
                                start=(j == 0), stop=(j == n_tt - 1),
                                skip_group_check=True,
                                tile_position=(0, 32 * h))

                    for j in range(n_tt):
                        jsl = slice(j * 128, (j + 1) * 128)
                        # two heads packed per S psum tile -> one exp op each;
                        # one PSUM bank per head (concurrent row-group matmuls
                        # into the same bank are a HW hazard)
                        pair_tiles = []
                        for pair in range(2):
                            s = sp.tile([128, 2, 512], F32, tag="s", name=f"s{pair}")
                            for k in range(2):
                                h = 2 * pair + k
                                hp = slice(32 * h, 32 * h + HS)
                                nc.tensor.matmul(
                                    s[:, k, 0:IC],
                                    lhsT=qT[b][hp, jsl],
                                    rhs=kT[b][hp, isl],
                                    start=True, stop=True,
                                    tile_position=(32 * h, 0))
                            pair_tiles.append(s)
                        es = []
                        for pair in range(2):
                            e = ep.tile([128, 2, 512], BF16, tag="e", name=f"e{pair}")
                            nc.scalar.activation(out=e[:, :, 0:IC],
                                                 in_=pair_tiles[pair][:, :, 0:IC],
                                                 func=AF.Exp)
                            es.append(e)
                        if last_pv[0] is not None:
                            last_pv[0]()
                        last_pv[0] = (lambda j=j, es=es, f=emit_pv: f(j, es))
                        # previous chunk's tail interleaves with this chunk's
                        # S/exp stream so ACT never drains at chunk boundaries
                        if j == min(2, n_tt - 1) and pending_c[0] is not None:
                            pending_c[0]()
                            pending_c[0] = None
                        # drip next batch's LN1/QKV into the stream
                        if gstep[0] % 2 == 1 and a_queue:
                            a_queue.pop(0)()
                        gstep[0] += 1
                    pending_c[0] = _make_stage_c(b, i0, pv)
            last_pv[0]()
            pending_c[0]()

    if split_waits:
        _split_multiwaits(nc)
    return nc


def _split_multiwaits(nc):
    """walrus codegen in this container encodes a limited number of sem
    waits per instruction (1 for Drain, 2 for compute ops); spill extras
    onto preceding NOPs on the same engine. DMA copies are left alone —
    their waits ride in the DGE descriptor."""
    for func in nc.m.functions:
        for bb in func.blocks:
            insts = list(bb.instructions)
            out, changed = [], False
            for ins in insts:
                si = ins.sync_info
                maxw = 1
                if (maxw is not None and si is not None and si.on_wait
                        and len(si.on_wait) > maxw):
                    waits = list(si.on_wait)
                    for k, w in enumerate(waits[:-maxw]):
                        nop = mybir.InstNoOp(
                            name=f"{ins.name}-wsplit{k}",
                            sync_info=mybir.SyncInfo(on_wait=[w], on_update=[]),
                            bass_nofuse=True, engine=ins.engine)
                        try:
                            nc.register_instruction(nop, overwrite=True)
                        except Exception:
                            pass
                        out.append(nop)
                    si.on_wait = waits[-maxw:]
                    changed = True
                out.append(ins)
            if changed:
                bb.instructions = out


_NC_CACHE = {}


def kernel(**inputs):
    from concourse.bass_utils import run_bass_kernel_spmd

    x = np.ascontiguousarray(np.asarray(inputs["x"], dtype=np.float32))
    b_full = x.shape[0]
    n_cores = N_CORES
    b_loc = b_full // n_cores

    key = (b_loc, x.shape[1])
    if key not in _NC_CACHE:
        _NC_CACHE[key] = build_kernel(b_loc, x.shape[1])
    nc = _NC_CACHE[key]

    weights = {k: np.ascontiguousarray(np.asarray(inputs[k], dtype=np.float32))
               for k in ("Wq", "Wk", "Wv", "Wp", "bp", "W1", "b1", "W2", "b2",
                         "g1", "be1", "g2", "be2")}
    in_maps = []
    for c in range(n_cores):
        m = {"x": x[c * b_loc : (c + 1) * b_loc]}
        m.update(weights)
        in_maps.append(m)

    res = run_bass_kernel_spmd(nc, in_maps, core_ids=list(range(n_cores)))
    out = np.concatenate([r["out"] for r in res.results], axis=0)
    return out
